# revision 5
# baseline (speedup 1.0000x reference)
"""Trainium2 Bass kernel for the seq2seq-style attention module.

Computation:
    score[s,b] = relu(enc[s,b,:]@w_enc + dec[b,:]@w_dec + bias)
    attn       = softmax(score, axis=s)
    out[b,:]   = sum_s attn[s,b] * enc[s,b,:]

Strategy (memory-bound: enc_states is 512MB, everything else tiny):
  * Data-parallel over batch: 8 cores x 4 batches. Core shard flattened to
    rows r = s*4 + b_local, shipped bf16 ([8192, 2048], 32MB, 93.2us DMA
    floor at the cost model's 360 B/ns; total here ~108.2us).
  * 64 blocks of 128 CONSECUTIVE rows; partition p holds one row,
    b_local(p) = p % 4. One contiguous 512KB DMA per block (128
    descriptors x 4KB, full bus rate). DMA order: w1, block0, cpack,
    blocks 1..63 -- the stream is gapless from ~2.0us to ~95.9us.
  * Per block: score column = enc_blk . w via a T/A/P engine mix
    (T: DVE fused multiply-reduce 2194ns; A: DVE mult 1127 + Act
    accumulate 2079; P: Pool mult 4158 + Act accumulate), balanced so
    DVE/Act/Pool each carry ~80us. Per group of 4 blocks: one Act exp
    (dec-dot rides the exp bias; exp(relu(x)) == max(exp(x),1) folds the
    relu into the a2 mask op), one DVE a2 build (mask[p, j*4+m] =
    (p%4==m)), then a PE burst of 4 context matmuls into 4 PSUM banks +
    a [128,4]x[128,1] denominator matmul per block. Group finishers are
    deferred one group so the a2 never head-of-line-blocks DVE.
  * PE p-state: the sim drops a sem-gated burst to the cold clock unless
    the PE was active within ~3us, so every block's DMA emits a 107ns
    "tick" matmul gated on its tile -- bursts always run at 213ns/matmul.
  * Tail: blocks 60-63 get chunked DMAs and split reductions (Pool/Act
    take half- and quarter-products in Act's late idle window, DVE fused
    ops s0-chain the partials) so the last 512-column chunk's reduce,
    exp, matmuls, PSUM evacuation and the single output DMA chase the
    final bytes.
  * Context ships UNNORMALIZED (denominator packed as column E); the
    host divides. Accuracy vs the fp32 reference: absmax-relative
    ~2.5e-3 (gate 2e-2), from the bf16 enc quantization.
"""

import os as _os

from contextlib import ExitStack

import ml_dtypes
import numpy as np

import concourse.bacc as bacc
import concourse.bass as bass
import concourse.mybir as mybir
import concourse.tile as tile
from concourse.bass_utils import run_bass_kernel_spmd
from concourse.dve_ops import TENSOR_TENSOR_REDUCE

S = 2048
B = 32
E = 2048
D = 1024
NCORES = 8
BPC = B // NCORES  # 4
P = 128
ROWS = S * BPC  # 8192
NBLK = ROWS // P  # 64
NB = E // 512  # psum banks
GRP = int(_os.environ.get("K_GRP", "4"))  # blocks per steady group
NSG = 60 // GRP  # steady groups -> blocks 0..59

F32 = mybir.dt.float32
BF16 = mybir.dt.bfloat16

# per-block reduce path for blocks 0..55 + 56..59:
#   T = DVE fused multiply-reduce (2351ns)
#   A = DVE multiply (1131) + Act accumulate (2412)
#   P = Pool multiply (4158) + Act accumulate (2412)
def _mk_paths(nt=24, na=19, np_=17):
    # Bresenham-spread the T/A/P mix over 60 blocks
    n = nt + na + np_
    counts = {"T": nt, "A": na, "P": np_}
    done = {"T": 0, "A": 0, "P": 0}
    out = []
    for i in range(n):
        k = max(counts, key=lambda c: counts[c] * (i + 1) / n - done[c])
        done[k] += 1
        out.append(k)
    return "".join(out)


PATHS = _os.environ.get("K_PATHS", _mk_paths())  # blocks 0..59

EBUFS = int(_os.environ.get("K_EBUFS", "20"))
TBUFS = int(_os.environ.get("K_TBUFS", "4"))
ABUFS = int(_os.environ.get("K_ABUFS", "3"))
GBUFS = int(_os.environ.get("K_GBUFS", "3"))
SBUFS = int(_os.environ.get("K_SBUFS", "8"))

DUMMY_G = int(_os.environ.get("K_DUMMY_G", "0"))  # before each steady burst
DUMMY_MID = int(_os.environ.get("K_DUMMY_MID", "0"))  # before blocks56-59 burst
A2E = _os.environ.get("K_A2E", "vec")
PIPE = int(_os.environ.get("K_PIPE", "1"))
DUMMY_T = tuple(
    int(x) for x in _os.environ.get("K_DUMMY_T", "0,0,0,0").split(",")
)  # before each tail block burst (gated on that block's tile)


def _build_module(dt_in):
    nc = bacc.Bacc(None, target_bir_lowering=False)

    # cpack per-partition constants: col0 dec bias f32; cols 1:33 mask64 as
    # bf16 pairs; col 33 ones as bf16 pair.
    w1 = nc.declare_dram_parameter("w1", [1, E], dt_in, isOutput=False)
    cpack = nc.declare_dram_parameter("cpack", [P, 34], F32, isOutput=False)
    enc = nc.declare_dram_parameter("enc", [ROWS, E], dt_in, isOutput=False)
    out = nc.declare_dram_parameter("out", [BPC, E + 1], F32, isOutput=True)

    with ExitStack() as ctx:
        tc = ctx.enter_context(tile.TileContext(nc))
        cpool = ctx.enter_context(tc.tile_pool(name="const", bufs=1))
        epool = ctx.enter_context(tc.tile_pool(name="enc", bufs=EBUFS))
        tpool = ctx.enter_context(tc.tile_pool(name="prodt", bufs=TBUFS))
        apool = ctx.enter_context(tc.tile_pool(name="proda", bufs=ABUFS))
        gpool = ctx.enter_context(tc.tile_pool(name="prodg", bufs=GBUFS))
        spool = ctx.enter_context(tc.tile_pool(name="stats", bufs=SBUFS))
        p5pool = ctx.enter_context(tc.tile_pool(name="prod5", bufs=4))
        tailp = ctx.enter_context(tc.tile_pool(name="tailp", bufs=1))
        psum = ctx.enter_context(
            tc.tile_pool(name="psum", bufs=1, space=bass.MemorySpace.PSUM)
        )

        # ---- DMA stream head: w1, enc block0, cpack, then blocks 1.. ----
        enc_tiles = [None] * NBLK

        def load_block(k):
            t = epool.tile([P, E], dt_in, name="enc_t")
            nc.sync.dma_start(t[:], enc[k * P : (k + 1) * P, :])
            enc_tiles[k] = t
            return t

        w1_t = cpool.tile([1, E], dt_in)
        nc.sync.dma_start(w1_t[:], w1[:])
        load_block(0)
        cp_t = cpool.tile([P, 34], F32)
        nc.sync.dma_start(cp_t[:], cpack[:])
        decb_t = cp_t[:, 0:1]
        mask32_t = cp_t[:, 1:33].bitcast(dt_in)  # [P, 64]
        ones_t = cp_t[:, 33:34].bitcast(dt_in)[:, 0:1]  # [P, 1]

        # broadcast w to 128 partitions on-chip: ones[1,128] x w[1,512] per
        # 512-chunk on the PE, PSUM->SBUF casts on DVE/Act.
        ones1 = cpool.tile([1, P], dt_in)
        nc.vector.memset(ones1[:], 1.0)
        wrep_t = cpool.tile([P, E], dt_in)
        wps = psum.tile([P, 2, 512], F32, name="wps")
        for c in range(4):
            h = c % 2
            nc.tensor.matmul(
                wps[:, h, :],
                lhsT=ones1[:],
                rhs=w1_t[:, c * 512 : (c + 1) * 512],
                start=True,
                stop=True,
            )
            if h == 0:
                nc.vector.tensor_scalar_mul(
                    wrep_t[:, c * 512 : (c + 1) * 512], wps[:, h, :], 1.0
                )
            else:
                nc.scalar.activation(
                    wrep_t[:, c * 512 : (c + 1) * 512],
                    wps[:, h, :],
                    mybir.ActivationFunctionType.Identity,
                )

        ctx_ps = [psum.tile([BPC, 512], F32, name=f"ctx_ps{n}") for n in range(NB)]
        l_ps = psum.tile([BPC, 1], F32, name="l_ps")
        ctx_sb = cpool.tile([BPC, E + 1], F32, name="ctx_sb")
        dum_ps = psum.tile([16, 512], F32, name="dum_ps")

        def emit_dummies(n, dep=None):
            # dependency-free (or dep-gated via rhs slice) PE ramp keepers,
            # 256-free: ~107ns ramped.
            rhs = wrep_t[:, 0:256] if dep is None else dep[:, 0:256]
            for _ in range(n):
                nc.tensor.matmul(
                    dum_ps[:, 0:256],
                    lhsT=mask32_t[:, 0:16],
                    rhs=rhs,
                    start=True,
                    stop=True,
                )

        def tick(ap):
            # one ~107ns dummy matmul gated on `ap`: keeps the PE's p-state
            # epoch alive (a gated burst only runs at full clock if the PE
            # was active within the last ~3us).
            nc.tensor.matmul(
                dum_ps[:, 0:256],
                lhsT=mask32_t[:, 0:16],
                rhs=ap[:, 0:256],
                start=True,
                stop=True,
            )

        def emit_reduce(enc_t, path, col):
            """col = enc_t[p,:] . w[:]  (no dec; dec rides the exp bias)."""
            tick(enc_t)
            if path == "T":
                prod = tpool.tile([P, E], dt_in, name="prod_t")
                nc.vector._custom_dve(
                    TENSOR_TENSOR_REDUCE,
                    out=prod[:],
                    in0=enc_t[:],
                    in1=wrep_t[:],
                    s0=0.0,
                    s1=1.0,
                    accum_out=col,
                )
            else:
                meng = nc.vector if path == "A" else nc.gpsimd
                pool_ = apool if path == "A" else gpool
                prod = pool_.tile([P, E], dt_in, name="prod_" + path)
                meng.tensor_mul(prod[:], enc_t[:], wrep_t[:])
                nc.scalar.activation(
                    prod[:],
                    prod[:],
                    mybir.ActivationFunctionType.Identity,
                    accum_out=col,
                )

        mm_first = [True]

        def emit_block_mms(enc_t, a2_4, stop=False):
            """denominator + 4 context matmuls for one block."""
            first = mm_first[0]
            mm_first[0] = False
            nc.tensor.matmul(
                l_ps[:], lhsT=a2_4, rhs=ones_t, start=first, stop=stop
            )
            for n in range(NB):
                nc.tensor.matmul(
                    ctx_ps[n][:],
                    lhsT=a2_4,
                    rhs=enc_t[:, n * 512 : (n + 1) * 512],
                    start=first,
                    stop=stop,
                )

        def emit_a2(a2, ecol, ngrp, veng):
            """a2[:, j*4+m] = (p%4==m) * max(ecol[:, j], 1)."""
            if veng is nc.gpsimd:
                # Pool-legal form: tensor_scalar_max + broadcast tensor_mul
                # (scalar_tensor_tensor fails the walrus engine check on Pool)
                ecol2 = spool.tile([P, ngrp], F32, name="ecol2")
                nc.gpsimd.tensor_scalar_max(ecol2[:], ecol[:], 1.0)
                nc.gpsimd.tensor_mul(
                    a2.rearrange("p (j m) -> p j m", j=ngrp),
                    mask32_t[:, 0 : ngrp * BPC].rearrange(
                        "p (j m) -> p j m", j=ngrp
                    ),
                    ecol2.unsqueeze(2).broadcast_to((P, ngrp, BPC)),
                )
            else:
                ecol_b = ecol.unsqueeze(2).broadcast_to((P, ngrp, BPC))
                veng.scalar_tensor_tensor(
                    out=a2.rearrange("p (j m) -> p j m", j=ngrp),
                    in0=ecol_b,
                    scalar=1.0,
                    in1=mask32_t[:, 0 : ngrp * BPC],
                    op0=mybir.AluOpType.max,
                    op1=mybir.AluOpType.mult,
                )

        def finish_group(pscore, blocks, ngrp, ndum):
            ecol = spool.tile([P, ngrp], F32, name="ecol")
            nc.scalar.activation(
                ecol[:],
                pscore[:],
                mybir.ActivationFunctionType.Exp,
                bias=decb_t,
            )
            a2 = spool.tile([P, ngrp * BPC], dt_in, name="a2")
            emit_a2(a2[:], ecol[:], ngrp, nc.gpsimd if A2E == "pool" else nc.vector)
            emit_dummies(ndum)
            for j, k in enumerate(blocks):
                emit_block_mms(enc_tiles[k], a2[:, j * BPC : (j + 1) * BPC])

        # ---- steady groups: blocks 0..59; finishers deferred one group so
        # the a2 op reaches the DVE queue after the next group's reduces
        # (no head-of-line park while waiting on the exp) ----
        pending = []
        for g in range(NSG):
            blocks = list(range(g * GRP, (g + 1) * GRP))
            pscore = spool.tile([P, GRP], F32, name="pscore")
            for j, k in enumerate(blocks):
                enc_t = enc_tiles[k] if k == 0 else load_block(k)
                emit_reduce(enc_t, PATHS[k], pscore[:, j : j + 1])
            pending.append((pscore, blocks))
            while len(pending) > PIPE:
                ps, bl = pending.pop(0)
                finish_group(ps, bl, GRP, DUMMY_G)
        while pending:
            ps, bl = pending.pop(0)
            finish_group(ps, bl, GRP, DUMMY_G)

        # ---- tail: blocks 60..63, chunked DMAs + split reductions.
        # DVE: b60 fused | b61h1 fused | b63 quarters fused | b62 halves
        # fused (last, carries stop flags). Pool: b61h0 mult + all the
        # partial adds + tail a2 builds. Act: acc61h0 + the four exps.
        H = 1024
        Q = 512

        def load_half(k, h):
            t = enc_tiles[k]
            nc.sync.dma_start(
                t[:, h * H : (h + 1) * H], enc[k * P : (k + 1) * P, h * H : (h + 1) * H]
            )

        load_block(60)
        for k in (61, 62):
            enc_tiles[k] = cpool.tile([P, E], dt_in, name=f"enc_t{k}")
            load_half(k, 0)
            load_half(k, 1)
        e63 = cpool.tile([P, E], dt_in, name="enc_t63")
        enc_tiles[63] = e63
        for q in range(4):
            nc.sync.dma_start(
                e63[:, q * Q : (q + 1) * Q],
                enc[63 * P : 64 * P, q * Q : (q + 1) * Q],
            )

        # separate [P,1] tiles per partial/final; partials chain through
        # the fused op's s0 init (same-engine, no cross-engine add hops)
        pt = {k: spool.tile([P, 1], F32, name=f"pt{k}") for k in range(60, 64)}
        pq = [spool.tile([P, 1], F32, name=f"pq{i}") for i in range(8)]

        def fused(sl, col, wsl, s0=0.0):
            tick(sl)
            prod = p5pool.tile([P, sl.shape[-1]], dt_in, name="prod_t5")
            nc.vector._custom_dve(
                TENSOR_TENSOR_REDUCE,
                out=prod[:],
                in0=sl,
                in1=wsl,
                s0=s0,
                s1=1.0,
                accum_out=col,
            )

        def mult(meng, name, k, h, width=H):
            prod = tailp.tile([P, width], dt_in, name=name)
            sl = enc_tiles[k][:, h * width : (h + 1) * width]
            wsl = wrep_t[:, h * width : (h + 1) * width]
            meng.tensor_mul(prod[:], sl, wsl)
            return prod

        def acc(prod, col):
            nc.scalar.activation(
                prod[:],
                prod[:],
                mybir.ActivationFunctionType.Identity,
                accum_out=col,
            )

        def fin_block(k, pcol, ndum, stop):
            j = k - 56
            ecol = spool.tile([P, 1], F32, name=f"ecol_t{j}")
            nc.scalar.activation(
                ecol[:], pcol, mybir.ActivationFunctionType.Exp, bias=decb_t
            )
            a2 = spool.tile([P, BPC], dt_in, name=f"a2_t{j}")
            emit_a2(a2[:], ecol[:], 1, A2ENG)
            emit_dummies(ndum, dep=enc_tiles[k])
            emit_block_mms(enc_tiles[k], a2[:], stop=stop)

        A2ENG = nc.gpsimd if A2E == "pool" else nc.vector
        w_h0, w_h1 = wrep_t[:, 0:H], wrep_t[:, H:E]

        TMODE = int(_os.environ.get("K_TMODE", "0"))
        if TMODE >= 1:
            # b60: DVE mult + Act acc (Act has late slack)
            p60 = mult(nc.vector, "prod60", 60, 0, width=E)
            acc(p60, pt[60][:])
        else:
            fused(enc_tiles[60][:], pt[60][:], wrep_t[:])
        fin_block(60, pt[60][:], 0, False)
        # b61: h0 Pool mult -> Act acc; h1 DVE (fused or mult+acc)
        p61h0 = mult(nc.gpsimd, "prod61h0", 61, 0)
        acc(p61h0, pq[1][:])
        if TMODE >= 2:
            p61h1 = mult(nc.vector, "prod61h1", 61, 1)
            acc(p61h1, pq[6][:])
            nc.gpsimd.tensor_add(pt[61][:], pq[1][:], pq[6][:])
        else:
            fused(enc_tiles[61][:, H:E], pt[61][:], w_h1, s0=pq[1][:])
        fin_block(61, pt[61][:], 0, False)
        # b62 h1: Pool mult early (data ~94.7); Act accumulates
        p62h1 = mult(nc.gpsimd, "prod62h1", 62, 1)
        acc(p62h1, pq[2][:])
        # b62 h0: DVE mult -> Act acc chained via s0? (acc overwrites) ->
        # DVE mult + Act acc to pq3, Pool add
        p62h0 = mult(nc.vector, "prod62h0", 62, 0)
        acc(p62h0, pq[3][:])
        nc.gpsimd.tensor_add(pt[62][:], pq[2][:], pq[3][:])
        fin_block(62, pt[62][:], 0, False)
        QMODE = int(_os.environ.get("K_QMODE", "0"))
        if QMODE:
            # all-DVE fused quarters, s0-chained: keeps Act's late queue
            # free for the exps (exp62 must not queue behind quarter accs)
            fused(e63[:, 0:Q], pq[4][:], wrep_t[:, 0:Q])
            fused(e63[:, Q : 2 * Q], pq[5][:], wrep_t[:, Q : 2 * Q], s0=pq[4][:])
        else:
            m0 = mult(nc.vector, "prod63q0", 63, 0, width=Q)
            acc(m0, pq[4][:])
            m1 = mult(nc.vector, "prod63q1", 63, 1, width=Q)
            acc(m1, pq[5][:])
        if TMODE == 4:
            m2 = mult(nc.gpsimd, "prod63q2", 63, 2, width=Q)
            acc(m2, pq[6][:])
            m3 = mult(nc.gpsimd, "prod63q3", 63, 3, width=Q)
            acc(m3, pq[7][:])
            nc.gpsimd.tensor_add(pq[4][:], pq[4][:], pq[5][:])
            nc.gpsimd.tensor_add(pq[6][:], pq[6][:], pq[7][:])
            nc.gpsimd.tensor_add(pt[63][:], pq[4][:], pq[6][:])
        elif TMODE >= 3:
            m2 = mult(nc.vector, "prod63q2", 63, 2, width=Q)
            acc(m2, pq[7][:])
            m3 = mult(nc.vector, "prod63q3", 63, 3, width=Q)
            nc.gpsimd.tensor_add(pq[4][:], pq[4][:], pq[5][:])
            acc(m3, pq[5][:])
            nc.gpsimd.tensor_add(pq[7][:], pq[7][:], pq[5][:])
            nc.gpsimd.tensor_add(pt[63][:], pq[4][:], pq[7][:])
        else:
            if QMODE:
                fused(e63[:, 2 * Q : 3 * Q], pq[6][:], wrep_t[:, 2 * Q : 3 * Q], s0=pq[5][:])
                fused(e63[:, 3 * Q : 4 * Q], pt[63][:], wrep_t[:, 3 * Q : 4 * Q], s0=pq[6][:])
            else:
                fused(e63[:, 2 * Q : 3 * Q], pq[6][:], wrep_t[:, 2 * Q : 3 * Q])
                fused(e63[:, 3 * Q : 4 * Q], pq[7][:], wrep_t[:, 3 * Q : 4 * Q], s0=pq[6][:])
                nc.gpsimd.tensor_add(pq[4][:], pq[4][:], pq[5][:])
                nc.gpsimd.tensor_add(pt[63][:], pq[7][:], pq[4][:])
        fin_block(63, pt[63][:], 0, True)

        # ---- evacuate PSUM banks (chasing the stop matmuls) + out ----
        nc.vector.tensor_scalar_mul(ctx_sb[:, E : E + 1], l_ps[:], 1.0)
        for n in range(NB):
            sl = ctx_sb[:, n * 512 : (n + 1) * 512]
            if n % 2 == 0:
                nc.scalar.activation(
                    sl, ctx_ps[n][:], mybir.ActivationFunctionType.Identity
                )
            else:
                nc.vector.tensor_scalar_mul(sl, ctx_ps[n][:], 1.0)
        nc.sync.dma_start(out[:], ctx_sb[:])

    nc.finalize()
    return nc


_CACHE = {}


def _get_module(dt_in):
    if dt_in not in _CACHE:
        _CACHE[dt_in] = _build_module(dt_in)
    return _CACHE[dt_in]


USE_BF16 = True


def _make_in_maps(dec_hidden, enc_states, W_energy, b_energy):
    np_in = ml_dtypes.bfloat16 if USE_BF16 else np.float32
    w = np.asarray(W_energy, np.float32)[0]
    w_dec, w_enc = w[:D], w[D:]
    dec_dot = (
        np.asarray(dec_hidden, np.float32)[0] @ w_dec + np.float32(b_energy[0])
    )  # [B]

    w1 = np.ascontiguousarray(w_enc.astype(np_in)).reshape(1, E)
    mask64 = np.zeros((P, 16 * BPC), np_in)
    for m in range(BPC):
        mask64[m::BPC, m::BPC] = 1.0

    enc = np.asarray(enc_states, np.float32)
    in_maps = []
    for c in range(NCORES):
        shard = np.ascontiguousarray(
            enc[:, c * BPC : (c + 1) * BPC, :], dtype=np_in
        ).reshape(ROWS, E)
        cpack = np.zeros((P, 34), np.float32)
        dec_c = dec_dot[c * BPC : (c + 1) * BPC].astype(np.float32)
        cpack[:, 0] = dec_c[np.arange(P) % BPC]
        cpack[:, 1:33].view(np.uint16)[:] = mask64.view(np.uint16)
        cpack[:, 33:34].view(np.uint16)[:, 0] = (
            np.float32(1.0).astype(ml_dtypes.bfloat16).view(np.uint16)
        )
        in_maps.append({"w1": w1, "cpack": cpack, "enc": shard})
    return in_maps


def kernel(dec_hidden, enc_states, W_energy, b_energy):
    dt_in = BF16 if USE_BF16 else F32
    nc = _get_module(dt_in)
    in_maps = _make_in_maps(dec_hidden, enc_states, W_energy, b_energy)
    res = run_bass_kernel_spmd(nc, in_maps, list(range(NCORES))).results
    ctx = np.empty((NCORES, BPC, E), np.float32)
    for c in range(NCORES):
        o = res[c]["out"]
        ctx[c] = o[:, :E] / o[:, E : E + 1]
    return ctx.reshape(1, B, E).astype(np.float32)
